# revision 46
# baseline (speedup 1.0000x reference)
"""ColBERT pairwise scoring kernel for 8x TRN2 NeuronCores.

Computation (see problem reference):
    qn = l2norm(q, axis=-1); kn = l2norm(k, axis=-1)
    S[b,o,i,j] = qn[b,i,:]·kn[o,j,:], masked positions -> -inf
    s[b,o] = sum_i logsumexp_j(ALPHA*S)/ALPHA, nonfinite -> 0
    out = s / (sqrt(Lq*Lk)+1e-6) * min(exp(logit_scale), 100)

Sharding: candidate axis O is split across the 8 cores (16 o's per core);
q is replicated. The reference zeroes every output row b whose q_mask has
any masked token, so only rows with no masked token are computed. Host
packs those nv rows (CBI = 32*nv query vectors), pre-normalizes q/k,
zeroes masked k rows, and ships everything fp8-e4m3 in a DoubleRow
(j,t)-interleaved layout as ONE buffer per core.

Device per core (fast path, nv <= 8):
  - 4 input DMAs (SP, Pool/SWDGE, ACT, SP queues) deliver qt + 16 o's of
    kt in compute order; transfers stream back-to-back on the DMA engines.
  - 64 fp8 DoubleRow matmuls (2 bi-half pieces per 128-j chunk) ->
    T [128, CBI/2] f32 pieces in PSUM, 512//(CBI/2) pieces per bank,
    banks 0..6 rotated (bank 7 = sums + warmup junk). Pieces deepen the
    bank-WAR rotation vs whole chunks, removing exp-engine stalls.
  - exp(ALPHA*T) via strided multi-bank sweeps that skip the bank pads:
    ACT runs the exact Exp activation, DVE runs a Schraudolph bf16
    bit-trick tensor_scalar; both write a packed bf16/int16 e tile in SBUF.
  - j-sums via stationary-e matmuls: lhsT = e-slice, rhs = ones -> one
    PSUM column per (chunk, bi-half); free-dim is 1 so these are ~free.
  - sums copied to SBUF and shipped with a pre-prepared SWDGE
    kv_writeback fired by trigger_dma (cheaper tail than a HWDGE DMA).
ln, Lq-sum, masked-count corrections, scaling and masking run on the
host. Since |ALPHA*S| <= 12.3, no max-subtraction is needed for a
stable logsumexp.

Fallback: nv == 0 -> all-zero output (host only); nv > 8 -> dense
program over all 2048 query rows (previous-generation schedule).
"""

import math
import sys
from contextlib import ExitStack

import numpy as np

for _p in ("/opt/trn_rl_repo",):
    if _p not in sys.path:
        sys.path.insert(0, _p)

import concourse.bass as bass
import concourse.bacc as bacc
import concourse.tile as tile
from concourse import mybir
from concourse.bass_utils import run_bass_kernel_spmd

ALPHA = 12.0
B, Lq, O, Lk, D = 64, 32, 128, 256, 128
NCORES = 8
OL = O // NCORES      # 16 candidates per core
KR = OL * Lk          # 4096 k rows per core
NCH = KR // 128       # 32 j-chunks per core
BI = B * Lq           # dense-path query rows

# DVE fast-exp (Schraudolph on bf16 bit patterns):
#   bf16_bits(e^y) ~= y * 128/ln(2) + (127*128 - C_CORR)
EXP_SLOPE = ALPHA * 184.66496234120901  # ALPHA * 2^7/ln2
C_CORR = 5.51
EXP_OFF = 16256.0 - C_CORR
V_DVE = 0.9765625  # bf16 value of int16 bits round(EXP_OFF) = 16250

F32 = mybir.dt.float32
F16 = mybir.dt.float16
BF16 = mybir.dt.bfloat16
I16 = mybir.dt.int16
I32 = mybir.dt.int32
F8 = mybir.dt.float8e4
AF = mybir.ActivationFunctionType
OP = mybir.AluOpType
DR = mybir.MatmulPerfMode.DoubleRow

# ---------------- fast path schedule (tuned against TimelineSim) -------------
FCFG = {
    # input DMA plan, in qk-column (== compute) order: (o_count, queue).
    # queues: "sp" (sync HWDGE), "act" (scalar HWDGE), "pool" (SWDGE).
    # Column order must match arrival order: HWDGE serves D0(sp) then the
    # act queue (its SEQ is free while sp is busy with D0) then the second
    # sp DMA; the pool transfer slots in right after D0's.
    "d_plan": [(3, "sp"), (4, "pool"), (6, "act"), (3, "sp")],
    # pieces mode: T is stored as half-chunk [128, CBI/2] pieces, 512//(CBI/2)
    # per PSUM bank (deeper WAR rotation); "sweeps" entries are then
    # (n_banks, engine) with 'A' (ACT exact exp) / 'D' (DVE Schraudolph),
    # summing to ceil(2*NCH/ppb) banks, no sweep wrapping past bank 6.
    "pieces": True,
    "sweeps": [(1, "A"), (1, "D"), (1, "D"), (2, "A"), (1, "A"), (1, "D"),
               (1, "D"), (2, "A"), (2, "D"), (1, "A")],
    "rlag": 1,    # sweeps of lag before the j-sum matmuls are emitted
    # partial S copies: (after_sweep_idx, col_lo, col_hi, engine)
    "copy_plan": [(7, 0, 44, "D"), (9, 44, 64, "D")],
    "nwarm": 12,
    "use_kv_out": True,
}

NBANK_CH = 2          # chunks per PSUM bank (within-bank matmul outputs)
ROT = 7 * NBANK_CH    # chunk rotation depth over banks 0..6


def _sweep_chunks(sweeps):
    """[(start_chunk, n_chunks, eng), ...] with window validation."""
    out = []
    c = 0
    for n, eng in sweeps:
        b0 = (c >> 1) % 7
        assert c % 2 == 0 and n % 2 == 0, (c, n)
        assert b0 + n // 2 <= 7, f"sweep at chunk {c} len {n} crosses rotation"
        out.append((c, n, eng))
        c += n
    assert c == NCH, c
    return out


def emit_fast(ctx, tc, qk_d, out_d, CBI, cfg):
    nc = tc.nc
    n_lo = min(CBI, 128)
    n_hi = CBI - n_lo

    sing = ctx.enter_context(tc.tile_pool(name="sing", bufs=1))
    pm = ctx.enter_context(tc.tile_pool(name="pm", bufs=1, space="PSUM"))

    NQ = CBI + KR          # per-t columns: q at n<CBI, k at n>=CBI
    QK = 2 * NQ
    qk = sing.tile([64, QK], F8)
    ECW = CBI  # e cols per chunk
    e16 = sing.tile([128, ECW * NCH], I16, name="e16")
    ones = sing.tile([128, 1], BF16)
    W0 = sing.tile([128, 64], BF16)
    ssb = sing.tile([128, 64], F32)
    idx0 = sing.tile([128, 1], I32)

    # one big PSUM tile: banks 0..6 hold T chunks (2 per bank at 192-col
    # slots), bank 7 holds S sums (cols 3584:3648) and warmup junk.
    T = pm.tile([128, 4096], F32, name="T")
    T3 = T.rearrange("p (b c) -> p b c", c=512)
    S = T[:, 3584:3648]
    junk = T[0:64, 3648:3712]

    ebf = e16.bitcast(BF16)

    # t-major views (matches DoubleRow Ldweights layout restrictions)
    qk2 = qk.rearrange("p (t n) -> p t n", t=2)
    qk2_d = qk_d.rearrange("p (t n) -> p t n", t=2)
    qtr = qk2[:, :, 0:CBI]

    def ktr_chunk(ch):
        return qk2[:, :, CBI + 128 * ch:CBI + 128 * (ch + 1)]

    # ---- memsets (DVE queue; idx0 must precede the kv prep) ----
    nc.vector.memset(ones, 1.0)
    nc.vector.memset(idx0, 0)
    nc.vector.memset(W0, 0.0)

    # ---- input DMAs in n (compute) order on their queues ----
    n0 = 0
    for n_o, q in cfg["d_plan"]:
        w = n_o * Lk + (CBI if n0 == 0 else 0)
        if q == "sp":
            eng = nc.sync
        elif q == "act":
            eng = nc.scalar
        elif q == "dve":
            eng = nc.vector
        else:
            eng = nc.gpsimd
        eng.dma_start(out=qk2[:, :, n0:n0 + w], in_=qk2_d[:, :, n0:n0 + w])
        n0 += w
    assert n0 == NQ, (n0, NQ)

    ssb4 = ssb.rearrange("p (a b n) -> p a b n", a=1, b=1)
    if cfg["use_kv_out"]:
        # Prep emitted EARLY so its ~1us Pool desc-gen overlaps the DMA
        # phase. Tile WAR-gates the later ssb writes against the deferred
        # DMA read; those spurious gates are zeroed post-compile
        # (_patch_kv_sync) — the trigger still RAW-waits the copies.
        dma_sem = nc.alloc_semaphore("swdge_out")
        nc.gpsimd.kv_writeback(out_d, ssb4, idx0, prepare_only=True,
                               sem=dma_sem)

    # ---- PE warmup junk matmuls ----
    for _ in range(cfg["nwarm"]):
        nc.tensor.matmul(out=junk, lhsT=W0, rhs=W0, start=True, stop=True,
                         skip_group_check=True)

    def emit_reduce(ch):
        c0 = ECW * ch
        nc.tensor.matmul(out=S[0:n_lo, 2 * ch:2 * ch + 1],
                         lhsT=ebf[:, c0:c0 + n_lo], rhs=ones,
                         start=True, stop=True, skip_group_check=True)
        if n_hi:
            nc.tensor.matmul(out=S[0:n_hi, 2 * ch + 1:2 * ch + 2],
                             lhsT=ebf[:, c0 + n_lo:c0 + CBI], rhs=ones,
                             start=True, stop=True, skip_group_check=True)

    def emit_exp(eng, in_ap, out_ap):
        if eng == "A":
            nc.scalar.activation(out=out_ap, in_=in_ap, func=AF.Exp,
                                 bias=0.0, scale=float(ALPHA))
        else:
            nc.vector.tensor_scalar(out=out_ap.bitcast(I16), in0=in_ap,
                                    scalar1=float(EXP_SLOPE),
                                    scalar2=float(EXP_OFF),
                                    op0=OP.mult, op1=OP.add)

    def emit_copy(lo, hi, engc):
        if engc == "D":
            nc.vector.tensor_copy(out=ssb[:, lo:hi], in_=S[:, lo:hi])
        else:
            nc.scalar.copy(out=ssb[:, lo:hi], in_=S[:, lo:hi])

    copy_by_sweep = {}
    for after, lo, hi, engc in cfg["copy_plan"]:
        copy_by_sweep.setdefault(after, []).append((lo, hi, engc))
    rlag = cfg["rlag"]

    if cfg.get("pieces"):
        # half-chunk pieces, ppb per PSUM bank: deeper rotation (7*ppb
        # pieces) than the 2-chunk/bank layout, so bank WAR stalls later.
        P = CBI // 2
        ppb = 512 // P
        NPC = 2 * NCH
        NBK = -(-NPC // ppb)
        sweeps = []   # (bank0, nbanks, pieces0, pieces1, eng)
        b = 0
        for nb, eng in cfg["sweeps"]:
            assert (b % 7) + nb <= 7, f"bank sweep at {b} len {nb} wraps"
            p0, p1 = ppb * b, min(ppb * (b + nb), NPC)
            if p1 - p0 < ppb * nb:
                assert nb == 1, "partial bank must be swept alone"
            sweeps.append((b, nb, p0, p1, eng))
            b += nb
        assert b == NBK, (b, NBK)

        def pc_cols(pc):
            return 512 * ((pc // ppb) % 7) + P * (pc % ppb)

        done_ch = [0]
        for step in range(len(sweeps) + rlag):
            if step < len(sweeps):
                b0, nb, p0, p1, eng = sweeps[step]
                for pc in range(p0, p1):
                    h = pc & 1
                    nc.tensor.matmul(
                        out=T[:, pc_cols(pc):pc_cols(pc) + P],
                        lhsT=ktr_chunk(pc >> 1),
                        rhs=qtr[:, :, h * P:(h + 1) * P],
                        start=True, stop=True, perf_mode=DR,
                        skip_group_check=True,
                    )
                width = (p1 - p0) // nb * P
                in_ap = T3[:, b0 % 7:b0 % 7 + nb, 0:width]
                out_ap = ebf[:, P * p0:P * p1].rearrange(
                    "p (b c) -> p b c", c=width)
                emit_exp(eng, in_ap, out_ap)
            r = step - rlag
            if r >= 0:
                p1 = sweeps[r][3]
                for ch in range(done_ch[0], p1 // 2):
                    emit_reduce(ch)
                done_ch[0] = p1 // 2
                for lo, hi, engc in copy_by_sweep.get(r, ()):
                    assert hi <= 2 * (sweeps[r][3] // 2), (r, lo, hi)
                    emit_copy(lo, hi, engc)
    else:
        def t_cols(ch):
            b = (ch >> 1) % 7
            s = ch & 1
            return 512 * b + CBI * s

        sweeps = _sweep_chunks(cfg["sweeps"])
        for step in range(len(sweeps) + rlag):
            if step < len(sweeps):
                c0, n, eng = sweeps[step]
                for ch in range(c0, c0 + n):
                    nc.tensor.matmul(
                        out=T[:, t_cols(ch):t_cols(ch) + CBI],
                        lhsT=ktr_chunk(ch),
                        rhs=qtr,
                        start=True, stop=True, perf_mode=DR,
                        skip_group_check=True,
                    )
                b0 = (c0 >> 1) % 7
                nb = n // 2
                in_ap = T3[:, b0:b0 + nb, 0:NBANK_CH * CBI]
                out_ap = ebf[:, ECW * c0:ECW * (c0 + n)].rearrange(
                    "p (b c) -> p b c", c=NBANK_CH * ECW)
                emit_exp(eng, in_ap, out_ap)
            r = step - rlag
            if r >= 0:
                pc0, pn, _ = sweeps[r]
                for ch in range(pc0, pc0 + pn):
                    emit_reduce(ch)
                for lo, hi, engc in copy_by_sweep.get(r, ()):
                    assert hi <= 2 * (pc0 + pn), (r, lo, hi)
                    emit_copy(lo, hi, engc)
    done_cols = max((hi for s, _, hi, _ in cfg["copy_plan"]
                     if s < len(sweeps)), default=0)
    if done_cols < 64:
        nc.vector.tensor_copy(out=ssb[:, done_cols:64],
                              in_=S[:, done_cols:64])

    if cfg["use_kv_out"]:
        # signals_writable=[ssb] makes the trigger a declared writer of
        # ssb, so Tile orders it after every copy into ssb (the DMA read
        # itself is deferred to the trigger and carries no RAW edge of its
        # own when the prep precedes the copies in program order).
        nc.gpsimd.trigger_dma(count=None, signals_writable=[ssb])
    else:
        nc.sync.dma_start(out=out_d, in_=ssb)


def _generic_cfg(CBI):
    """Fallback schedule for CBI != 192 (any nv <= 8): alternate A/D over
    bank sweeps, last (possibly partial) bank swept alone."""
    import copy as _copy
    cfg = _copy.deepcopy(FCFG)
    P = CBI // 2
    ppb = 512 // P
    NBK = -(-2 * NCH // ppb)
    sweeps = []
    b = 0
    i = 0
    while b < NBK:
        n = 1 if (b == NBK - 1 or (b % 7) == 6) else min(2, NBK - 1 - b)
        n = max(1, min(n, 7 - (b % 7)))
        sweeps.append((n, "AD"[i % 2]))
        b += n
        i += 1
    cfg["sweeps"] = sweeps
    cum = []
    t = 0
    for n, _ in sweeps:
        t += n
        cum.append(t)
    j = max(0, len(sweeps) - 2)
    hi = min(2 * (min(ppb * cum[j - 1] if j else 0, 2 * NCH) // 2), 56)
    cfg["copy_plan"] = [(max(0, j - 1), 0, hi, "D"),
                        (len(sweeps) - 1, hi, 64, "D")]
    return cfg


def build_fast(CBI, cfg=None):
    cfg = cfg or (FCFG if CBI == 192 else _generic_cfg(CBI))
    nc = bacc.Bacc("TRN2", target_bir_lowering=False, debug=False,
                   enable_asserts=False, num_devices=NCORES)
    QK = 2 * (CBI + KR)
    qk_d = nc.dram_tensor("qk_in", [64, QK], F8, kind="ExternalInput").ap()
    oshape = [1, 128, 1, 64] if cfg["use_kv_out"] else [128, 64]
    out_d = nc.dram_tensor("outp", oshape, F32, kind="ExternalOutput").ap()
    with tile.TileContext(nc) as tc, ExitStack() as ctx:
        emit_fast(ctx, tc, qk_d, out_d, CBI, cfg)
    nc.compile()
    if cfg["use_kv_out"]:
        _retarget_prep_sem(nc)
    return nc


def _retarget_prep_sem(nc):
    """Fix up the SWDGE prepare/trigger sync for the kv_writeback output.

    1. Point the prep's DMA-completion update at the Tile-assigned DMASW
       lane sem: the teardown waits on the lane sem, but the prepare_only
       path bakes the user-provided sem into the descriptor, so the lane
       wait would never be satisfied. The lane sem is the unique DMASW*
       sem that is waited on but never updated.
    2. Zero the body-block waits on that lane sem: they are WAR gates
       ordering the ssb copies after the (deferred) DMA read, but the DMA
       only fires at the trigger, which RAW-waits those very copies — the
       gates would deadlock. The teardown-block waits are kept so the
       program still drains the DMA."""
    fn = nc.m.functions[0]
    wait_ids = {}
    upd_ids = set()
    preps = []
    blocks = list(fn.blocks)
    for bb in blocks:
        for i in bb.instructions:
            si = i.sync_info
            if i.opcode == "KVWritebackAnt" and getattr(i, "gen_mode", 0) == 1:
                preps.append(i)
            if si is None:
                continue
            for w in (si.on_wait or []):
                nm = getattr(w, "ant_name", "") or ""
                if nm.startswith("DMASW"):
                    wait_ids[w.id] = nm
            for u in (si.on_update or []):
                upd_ids.add(u.id)
    orphans = [k for k in wait_ids if k not in upd_ids]
    assert len(preps) == 1 and len(orphans) == 1, (preps, orphans, wait_ids)
    lane = orphans[0]
    preps[0].sync_info.on_update[0].id = lane
    for bb in blocks[:-1]:
        for i in bb.instructions:
            si = i.sync_info
            if si is None:
                continue
            for w in (si.on_wait or []):
                if w.id == lane and (getattr(w, "ant_name", "") or "").startswith("DMASW"):
                    w.wait_value = 0


def make_in_maps_fast(q, k, k_mask, valid_idx, CBI):
    import ml_dtypes
    F8NP = ml_dtypes.float8_e4m3

    qf = np.asarray(q, dtype=np.float32).reshape(B, Lq, D)
    qsel = np.zeros((CBI, D), dtype=np.float32)
    nv = len(valid_idx)
    qn = qf[valid_idx].reshape(-1, D)
    qn = qn / np.maximum(np.sqrt((qn * qn).sum(-1, keepdims=True)), 1e-12)
    qsel[:nv * Lq] = qn

    kf = np.asarray(k, dtype=np.float32).reshape(O * Lk, D)
    kn = kf / np.maximum(np.sqrt((kf * kf).sum(-1, keepdims=True)), 1e-12)
    km = np.asarray(k_mask).astype(bool).reshape(O * Lk)
    kn[km] = 0.0

    in_maps = []
    for c in range(NCORES):
        kc = kn[c * KR:(c + 1) * KR]
        # [n, t, p] with q at n<CBI, k at n>=CBI; flatten t-major per p
        full = np.concatenate([qsel.reshape(CBI, 2, 64),
                               kc.reshape(KR, 2, 64)], axis=0)
        qk8 = np.ascontiguousarray(
            full.transpose(2, 1, 0).reshape(64, 2 * (CBI + KR))
        ).astype(F8NP)
        in_maps.append({"qk_in": qk8})
    return in_maps


def postprocess_fast(per_core_out, valid_idx, q_mask, k_mask, logit_scale,
                     CBI, cfg=None):
    """per-core out [1,128,1,64] -> [128, 64]: col 2ch = bi 0:128 sums,
    col 2ch+1 = bi 128:CBI sums (chunk ch = o_local*2 + jc)."""
    cfg = cfg or FCFG
    n_lo = min(CBI, 128)
    n_hi = CBI - n_lo
    nv = len(valid_idx)
    # masked-k exp contribution per (chunk, bi): 1.0 where ACT computed the
    # exact exp, V_DVE where the DVE bit-trick ran. In pieces mode each
    # chunk's two bi-halves (CBI/2 each) can land in different sweeps.
    vbi = np.empty((CBI, NCH), dtype=np.float64)
    if cfg.get("pieces"):
        P = CBI // 2
        ppb = 512 // P
        veng_piece = np.empty(2 * NCH, dtype=np.float64)
        b = 0
        for nb, eng in cfg["sweeps"]:
            p0, p1 = ppb * b, min(ppb * (b + nb), 2 * NCH)
            veng_piece[p0:p1] = 1.0 if eng == "A" else V_DVE
            b += nb
        for ch in range(NCH):
            vbi[:P, ch] = veng_piece[2 * ch]
            vbi[P:, ch] = veng_piece[2 * ch + 1]
    else:
        c = 0
        for n, eng in cfg["sweeps"]:
            vbi[:, c:c + n] = 1.0 if eng == "A" else V_DVE
            c += n
    kmc = np.asarray(k_mask).astype(bool).reshape(O, 2, 128).sum(-1)  # [O,jc]

    s = np.zeros((B, O), dtype=np.float32)
    with np.errstate(divide="ignore", invalid="ignore"):
        for core in range(NCORES):
            r = np.asarray(per_core_out[core]).reshape(128, 64)
            # sums[bi, ch]
            sums = np.empty((CBI, NCH), dtype=np.float64)
            for ch in range(NCH):
                sums[:n_lo, ch] = r[:n_lo, 2 * ch]
                if n_hi:
                    sums[n_lo:CBI, ch] = r[:n_hi, 2 * ch + 1]
                o_g = core * OL + (ch >> 1)
                sums[:, ch] -= kmc[o_g, ch & 1] * vbi[:, ch]
            tot = sums.reshape(CBI, OL, 2).sum(axis=2)  # [bi, o_local]
            lse = np.log(np.maximum(tot, 1e-30))
            sd = lse.reshape(CBI // Lq, Lq, OL).sum(axis=1)  # [vb, o_local]
            s[valid_idx, core * OL:(core + 1) * OL] = sd[:nv]
    coef = min(math.exp(float(logit_scale)), 100.0) / (
        ALPHA * (math.sqrt(Lq * Lk) + 1e-06))
    s = s * np.float32(coef)
    s[:, np.asarray(k_mask).astype(bool).all(axis=1)] = 0.0
    s = np.where(np.isfinite(s), s, 0.0).astype(np.float32)
    return s


# ======================= dense fallback (nv > 8) ============================
CFG = {
    "dve_exp": frozenset(range(1, 64, 2)),
    "plain_lag": 4,
    "plain_lag_tail": 3,
    "nwarm": 8,
}
DVE_EXP = CFG["dve_exp"]


def emit_dense(ctx, tc, qt_d, kt_d, out_d):
    nc = tc.nc
    NIT = NCH * 2

    sing = ctx.enter_context(tc.tile_pool(name="sing", bufs=1))
    epool = ctx.enter_context(tc.tile_pool(name="epool", bufs=7))
    edpool = ctx.enter_context(tc.tile_pool(name="edpool", bufs=6))
    pm = ctx.enter_context(tc.tile_pool(name="pm", bufs=3, space="PSUM"))
    plp = ctx.enter_context(tc.tile_pool(name="plp", bufs=1, space="PSUM"))
    wp = ctx.enter_context(tc.tile_pool(name="wp", bufs=1, space="PSUM"))

    qt = sing.tile([64, 2 * BI], F8)
    kt = sing.tile([64, 2 * KR], F8)
    W = sing.tile([128, 256], BF16)
    ssum = sing.tile([128, 256], F32)
    qtr = qt.rearrange("p (t n) -> p t n", t=2)
    ktr = kt.rearrange("p (t n) -> p t n", t=2)

    qt3_d = qt_d.rearrange("p (t n) -> p t n", t=2)
    kt3_d = kt_d.rearrange("p (t n) -> p t n", t=2)
    nc.sync.dma_start(out=ktr[:, :, 0:256], in_=kt3_d[:, :, 0:256])
    nc.scalar.dma_start(out=qtr[:, :, 0:1024], in_=qt3_d[:, :, 0:1024])
    nc.sync.dma_start(out=qtr[:, :, 1024:2048], in_=qt3_d[:, :, 1024:2048])
    nc.scalar.dma_start(out=ktr[:, :, 256:2048], in_=kt3_d[:, :, 256:2048])
    nc.sync.dma_start(out=ktr[:, :, 2048:4096], in_=kt3_d[:, :, 2048:4096])

    nc.vector.memset(W, 0.0)
    nc.vector.memset(W[:, 128:129], 1.0)

    plse = plp.tile([128, 256], F32)
    junk = wp.tile([128, 128], F32)
    for _ in range(CFG["nwarm"]):
        nc.tensor.matmul(out=junk, lhsT=W[:, 0:128], rhs=W[:, 0:128],
                         start=True, stop=True, skip_group_check=True)

    due = {}
    n_units = 0
    for p in range(NIT):
        lag = CFG["plain_lag"] if p < NIT - 6 else CFG["plain_lag_tail"]
        due.setdefault(p + lag, []).append((p % 2, p))
        n_units += 1

    Tt = {}
    et = {}
    n_done = 0
    last_index = max(due)
    for it in range(last_index + 1):
        if it < NIT:
            ch = it // 2
            h = it % 2
            T = pm.tile([128, 1024], F32, tag="mm")
            for sx in range(2):
                nc.tensor.matmul(
                    out=T[:, sx * 512:(sx + 1) * 512],
                    lhsT=ktr[:, :, ch * 128:(ch + 1) * 128],
                    rhs=qtr[:, :, h * 1024 + sx * 512: h * 1024 + (sx + 1) * 512],
                    start=True, stop=True, perf_mode=DR,
                )
            Tt[it] = T
        if 0 < it <= NIT:
            p = it - 1
            T = Tt.pop(p)
            if p in DVE_EXP:
                ed = edpool.tile([128, 1024], I16, tag="ed")
                nc.vector.tensor_scalar(
                    out=ed, in0=T, scalar1=float(EXP_SLOPE),
                    scalar2=float(EXP_OFF), op0=OP.mult, op1=OP.add)
                et[p] = ed.bitcast(BF16)
            else:
                e = epool.tile([128, 1024], BF16, tag="e")
                nc.scalar.activation(out=e, in_=T, func=AF.Exp,
                                     bias=0.0, scale=float(ALPHA))
                et[p] = e
        for h, p in due.get(it, ()):
            o = p // 4
            e = et.pop(p)
            for hb in range(4):
                pp = o * 8 + h * 4 + hb
                n_done += 0 if hb else 1
                nc.tensor.matmul(
                    out=plse,
                    lhsT=W[:, 128 - pp:256 - pp],
                    rhs=e[:, hb * 256:(hb + 1) * 256],
                    start=(n_done == 1 and hb == 0),
                    stop=(n_done == n_units and hb == 3),
                )

    nc.vector.tensor_copy(out=ssum, in_=plse)
    nc.scalar.dma_start(out=out_d, in_=ssum)


def build_dense():
    nc = bacc.Bacc("TRN2", target_bir_lowering=False, debug=False,
                   enable_asserts=False, num_devices=NCORES)
    qt_d = nc.dram_tensor("qt_in", [64, 2 * BI], F8, kind="ExternalInput").ap()
    kt_d = nc.dram_tensor("kt_in", [64, 2 * KR], F8, kind="ExternalInput").ap()
    out_d = nc.dram_tensor("outp", [128, 256], F32, kind="ExternalOutput").ap()
    with tile.TileContext(nc) as tc, ExitStack() as ctx:
        emit_dense(ctx, tc, qt_d, kt_d, out_d)
    nc.compile()
    return nc


def make_in_maps_dense(q, k, k_mask):
    import ml_dtypes
    F8NP = ml_dtypes.float8_e4m3

    qf = np.asarray(q, dtype=np.float32).reshape(BI, D)
    qn = qf / np.maximum(np.sqrt((qf * qf).sum(-1, keepdims=True)), 1e-12)
    qt8 = np.ascontiguousarray(
        qn.T.reshape(2, 64, BI).transpose(1, 0, 2).reshape(64, 2 * BI)
    ).astype(F8NP)

    kf = np.asarray(k, dtype=np.float32).reshape(O * Lk, D)
    kn = kf / np.maximum(np.sqrt((kf * kf).sum(-1, keepdims=True)), 1e-12)
    km = np.asarray(k_mask).astype(bool).reshape(O * Lk)
    kn[km] = 0.0
    ktf = kn.T.reshape(2, 64, O * Lk).transpose(1, 0, 2)

    in_maps = []
    for c in range(NCORES):
        kt8 = np.ascontiguousarray(
            ktf[:, :, c * KR:(c + 1) * KR].reshape(64, 2 * KR)
        ).astype(F8NP)
        in_maps.append({"qt_in": qt8, "kt_in": kt8})
    return in_maps


def postprocess_dense(per_core_out, q_mask, k_mask, logit_scale):
    kmc = np.asarray(k_mask).astype(bool).reshape(O, 2, 128).sum(-1)
    corr = np.zeros((O, 2), dtype=np.float64)
    for ol in range(OL):
        for jc in range(2):
            for h in range(2):
                it = (ol * 2 + jc) * 2 + h
                v = V_DVE if it in DVE_EXP else 1.0
                for c in range(NCORES):
                    corr[c * OL + ol, h] += kmc[c * OL + ol, jc] * v
    s = np.empty((B, O), dtype=np.float32)
    with np.errstate(divide="ignore", invalid="ignore"):
        for c in range(NCORES):
            r = np.array(per_core_out[c]).reshape(OL, 8, 256)
            r = r.reshape(OL, 8, 8, Lq)
            cc = corr[c * OL:(c + 1) * OL].reshape(OL, 2, 1, 1, 1)
            rr = r.reshape(OL, 2, 4, 8, Lq) - cc
            lse = np.log(np.maximum(rr.reshape(OL, 8, 8, Lq), 1e-30))
            sd = lse.sum(axis=3).reshape(OL, B)
            s[:, c * OL:(c + 1) * OL] = sd.T
    coef = min(math.exp(float(logit_scale)), 100.0) / (
        ALPHA * (math.sqrt(Lq * Lk) + 1e-06))
    s = s * np.float32(coef)
    s[np.asarray(q_mask).astype(bool).any(axis=1), :] = 0.0
    s[:, np.asarray(k_mask).astype(bool).all(axis=1)] = 0.0
    s = np.where(np.isfinite(s), s, 0.0).astype(np.float32)
    return s


_CACHED = {}
_LAST_NC = None
_LAST_IN_MAPS = None


def kernel(q, k, q_mask, k_mask, logit_scale):
    global _LAST_NC, _LAST_IN_MAPS
    qm = np.asarray(q_mask).astype(bool)
    valid_idx = np.nonzero(~qm.any(axis=1))[0]
    nv = len(valid_idx)
    if nv == 0:
        return np.zeros((B, O), dtype=np.float32)
    if nv <= 8:
        # CBI=32 (nv=1) trips a BIR access-pattern count limit; pad to 64
        CBI = 32 * max(nv, 2)
        cfg = FCFG if CBI == 192 else _generic_cfg(CBI)
        key = ("fast", CBI)
        if key not in _CACHED:
            _CACHED[key] = build_fast(CBI, cfg)
        nc = _CACHED[key]
        in_maps = make_in_maps_fast(np.asarray(q), np.asarray(k),
                                    np.asarray(k_mask), valid_idx, CBI)
        _LAST_NC, _LAST_IN_MAPS = nc, in_maps
        res = run_bass_kernel_spmd(nc, in_maps, list(range(NCORES)))
        outs = [np.asarray(res.results[c]["outp"]) for c in range(NCORES)]
        return postprocess_fast(outs, valid_idx, q_mask, k_mask, logit_scale,
                                CBI, cfg)
    key = "dense"
    if key not in _CACHED:
        _CACHED[key] = build_dense()
    nc = _CACHED[key]
    in_maps = make_in_maps_dense(np.asarray(q), np.asarray(k),
                                 np.asarray(k_mask))
    _LAST_NC, _LAST_IN_MAPS = nc, in_maps
    res = run_bass_kernel_spmd(nc, in_maps, list(range(NCORES)))
    outs = [np.asarray(res.results[c]["outp"]) for c in range(NCORES)]
    return postprocess_dense(outs, q_mask, k_mask, logit_scale)
